# revision 19
# baseline (speedup 1.0000x reference)
"""BinaryDense Trainium2 kernel: out = x @ sign(kernel) + bias.

Shapes (hardcoded): x [8192, 4096] f32, kernel [4096, 4096] f32,
bias [4096] f32 -> out [8192, 4096] f32.

Strategy: data-parallel over the 8 NeuronCores -- each core owns a
1024-row slice of x and the full weight matrix.  The x slice is staged
into device DRAM K-major (transposed during host-side sharding, a pure
layout choice) so the contraction dim lands on SBUF partitions without
any on-device transpose.  Per core:
  1. x^T streams in once (fp32), is cast to fp16 on the Vector engine,
     and lives in a persistent SBUF cache [K=4096, 1024] fp16.  fp16
     keeps ~1e-4 relative error for this problem (sign weights are
     exactly +-1 in fp16; only x is rounded) and enables Fast Weight
     Load, which keeps the PE at its 216 ns/matmul issue floor.
  2. Weights stream in [128, 512] f32 tiles; sign() runs on the Scalar
     engine writing fp16.
  3. 8 PSUM banks accumulate the 8 row-tiles of an output column block
     over the 32 k-chunks; DVE adds bias; results DMA out on the
     Activation-engine HWDGE queue, off the input-stream queue.

The first column-block pass is DMA-bound (first-touch of x^T + its
weight slice needs ~27 MB against ~350 GB/s per-core HBM), so the x^T
chunk DMAs are interleaved just-in-time, 4 chunks ahead of matmul
consumption, ahead of the weight tile of the same k step.
"""

import numpy as np
from contextlib import ExitStack

import concourse.bass as bass
import concourse.mybir as mybir
import concourse.tile as tile
from concourse import bacc
from concourse.bass import ts
from concourse.bass_utils import run_bass_kernel_spmd

B, D_IN, UNITS = 8192, 4096, 4096
N_CORES = 8
ROWS = B // N_CORES  # 1024 rows of x per core

P = 128
N_TILE = 512  # output-column tile (one PSUM bank of f32)

F32 = mybir.dt.float32
F16 = mybir.dt.float16


def build_body(tc, xt_dram, w, bias, out, rows, d_in, units, n_tile=N_TILE):
    nc = tc.nc
    b_tiles = rows // P
    k_tiles = d_in // P
    u_tiles = units // n_tile

    with ExitStack() as ctx:
        const = ctx.enter_context(tc.tile_pool(name="const", bufs=1))
        xt_pool = ctx.enter_context(tc.tile_pool(name="xt", bufs=1))
        stg = ctx.enter_context(tc.tile_pool(name="stg", bufs=8))
        wp = ctx.enter_context(tc.tile_pool(name="wp", bufs=8))
        sp = ctx.enter_context(tc.tile_pool(name="sp", bufs=8))
        op = ctx.enter_context(tc.tile_pool(name="op", bufs=8))

        bias_bc = const.tile([P, units], F32)

        # Persistent x^T cache (fp16). xt_dram rows are k; row
        # ko*128+ki -> partition ki, free (ko, b).
        xt = xt_pool.tile([P, k_tiles, rows], F16)
        xt_src = xt_dram.rearrange("(ko ki) b -> ki ko b", ki=P)

        def load_xt(ko):
            s = stg.tile([P, rows], F32, tag="stg")
            nc.sync.dma_start(s[:], xt_src[:, ko, :])
            nc.vector.tensor_copy(xt[:, ko, :], s[:])

        with tc.tile_pool(name="mpsum", bufs=b_tiles, space="PSUM") as mpsum:
            for u in range(u_tiles):
                psums = [
                    mpsum.tile([P, n_tile], F32, tag="acc", name=f"acc_{u}_{i}")
                    for i in range(b_tiles)
                ]
                for kc in range(k_tiles):
                    wt = wp.tile([P, n_tile], F32, tag="wt")
                    if u == 0 and kc == 0:
                        # first weight tile ahead of everything: each DMA
                        # trigger costs ~620ns of Sync-engine time and the
                        # first matmul is gated on sign(W[0,0])
                        nc.sync.dma_start(wt[:], w[ts(kc, P), ts(u, n_tile)])
                        for ko in range(min(2, k_tiles)):
                            load_xt(ko)
                    else:
                        if u == 0 and kc + 1 < k_tiles:
                            load_xt(kc + 1)
                        nc.sync.dma_start(wt[:], w[ts(kc, P), ts(u, n_tile)])
                    if kc == max(k_tiles - 4, 0):
                        # this u's bias slice, shortly before its drain
                        nc.sync.dma_start(
                            bias_bc[:, ts(u, n_tile)],
                            bias[None, ts(u, n_tile)].to_broadcast([P, n_tile]),
                        )
                    st = sp.tile([P, n_tile], F16, tag="st")
                    nc.scalar.activation(
                        st[:], wt[:], mybir.ActivationFunctionType.Sign
                    )
                    for bt in range(b_tiles):
                        nc.tensor.matmul(
                            psums[bt][:],
                            xt[:, kc, ts(bt, P)],
                            st[:],
                            start=(kc == 0),
                            stop=(kc == k_tiles - 1),
                        )
                for bt in range(b_tiles):
                    ot = op.tile([P, n_tile], F32, tag="ot")
                    nc.vector.tensor_add(
                        ot[:], psums[bt][:], bias_bc[:, ts(u, n_tile)]
                    )
                    nc.scalar.dma_start(out[ts(bt, P), ts(u, n_tile)], ot[:])


def build_nc(rows=ROWS, d_in=D_IN, units=UNITS, n_tile=N_TILE):
    nc = bacc.Bacc(
        "TRN2", target_bir_lowering=False, debug=False, num_devices=N_CORES
    )
    xt = nc.dram_tensor("xt", [d_in, rows], F32, kind="ExternalInput").ap()
    w = nc.dram_tensor("w", [d_in, units], F32, kind="ExternalInput").ap()
    bias = nc.dram_tensor("bias", [units], F32, kind="ExternalInput").ap()
    out = nc.dram_tensor("out", [rows, units], F32, kind="ExternalOutput").ap()
    with tile.TileContext(nc) as tc:
        build_body(tc, xt, w, bias, out, rows, d_in, units, n_tile)
    nc.compile()
    return nc


_NC = None


def _get_nc():
    global _NC
    if _NC is None:
        _NC = build_nc()
    return _NC


def run_spmd(x, w, b, trace=False):
    nc = _get_nc()
    in_maps = [
        {
            "xt": np.ascontiguousarray(x[c * ROWS : (c + 1) * ROWS].T),
            "w": w,
            "bias": b,
        }
        for c in range(N_CORES)
    ]
    res = run_bass_kernel_spmd(
        nc, in_maps, core_ids=list(range(N_CORES)), trace=trace
    )
    out = np.concatenate([res.results[c]["out"] for c in range(N_CORES)], axis=0)
    return out, res


def kernel(x, kernel, bias):
    x = np.ascontiguousarray(x, dtype=np.float32)
    w = np.ascontiguousarray(kernel, dtype=np.float32)
    b = np.ascontiguousarray(bias, dtype=np.float32)
    out, _ = run_spmd(x, w, b)
    return out


# revision 20
# speedup vs baseline: 1.0081x; 1.0081x over previous
"""BinaryDense Trainium2 kernel: out = x @ sign(kernel) + bias.

Shapes (hardcoded): x [8192, 4096] f32, kernel [4096, 4096] f32,
bias [4096] f32 -> out [8192, 4096] f32.

Strategy: data-parallel over the 8 NeuronCores -- each core owns a
1024-row slice of x and the full weight matrix.  The x slice is staged
into device DRAM K-major (transposed during host-side sharding, a pure
layout choice) so the contraction dim lands on SBUF partitions without
any on-device transpose.  Per core:
  1. x^T streams in once (fp32), is cast to fp16 on the Vector engine,
     and lives in a persistent SBUF cache [K=4096, 1024] fp16.  fp16
     keeps ~1e-4 relative error for this problem (sign weights are
     exactly +-1 in fp16; only x is rounded) and enables Fast Weight
     Load, which keeps the PE at its 216 ns/matmul issue floor.
  2. Weights stream in [128, 512] f32 tiles; sign() runs on the Scalar
     engine writing fp16.
  3. 8 PSUM banks accumulate the 8 row-tiles of an output column block
     over the 32 k-chunks; DVE adds bias; results DMA out on the
     Activation-engine HWDGE queue, off the input-stream queue.

The first column-block pass is DMA-bound (first-touch of x^T + its
weight slice needs ~27 MB against ~350 GB/s per-core HBM), so the x^T
chunk DMAs are interleaved just-in-time, 4 chunks ahead of matmul
consumption, ahead of the weight tile of the same k step.
"""

import numpy as np
from contextlib import ExitStack

import concourse.bass as bass
import concourse.mybir as mybir
import concourse.tile as tile
from concourse import bacc
from concourse.bass import ts
from concourse.bass_utils import run_bass_kernel_spmd

B, D_IN, UNITS = 8192, 4096, 4096
N_CORES = 8
ROWS = B // N_CORES  # 1024 rows of x per core

P = 128
N_TILE = 512  # output-column tile (one PSUM bank of f32)

F32 = mybir.dt.float32
F16 = mybir.dt.float16


def build_body(tc, xt_dram, w, bias, out, rows, d_in, units, n_tile=N_TILE):
    nc = tc.nc
    b_tiles = rows // P
    k_tiles = d_in // P
    u_tiles = units // n_tile

    with ExitStack() as ctx:
        const = ctx.enter_context(tc.tile_pool(name="const", bufs=1))
        xt_pool = ctx.enter_context(tc.tile_pool(name="xt", bufs=1))
        stg = ctx.enter_context(tc.tile_pool(name="stg", bufs=8))
        wp = ctx.enter_context(tc.tile_pool(name="wp", bufs=8))
        sp = ctx.enter_context(tc.tile_pool(name="sp", bufs=8))
        op = ctx.enter_context(tc.tile_pool(name="op", bufs=8))

        bias_bc = const.tile([P, units], F32)

        # Persistent x^T cache (fp16). xt_dram rows are k; row
        # ko*128+ki -> partition ki, free (ko, b).
        xt = xt_pool.tile([P, k_tiles, rows], F16)
        xt_src = xt_dram.rearrange("(ko ki) b -> ki ko b", ki=P)

        def load_xt(ko):
            s = stg.tile([P, rows], F32, tag="stg")
            nc.sync.dma_start(s[:], xt_src[:, ko, :])
            nc.vector.tensor_copy(xt[:, ko, :], s[:])

        with tc.tile_pool(name="mpsum", bufs=b_tiles, space="PSUM") as mpsum:
            for u in range(u_tiles):
                psums = [
                    mpsum.tile([P, n_tile], F32, tag="acc", name=f"acc_{u}_{i}")
                    for i in range(b_tiles)
                ]
                for kc in range(k_tiles):
                    wt = wp.tile([P, n_tile], F32, tag="wt")
                    if u == 0 and kc == 0:
                        # first weight tile ahead of everything: each DMA
                        # trigger costs ~620ns of Sync-engine time and the
                        # first matmul is gated on sign(W[0,0])
                        nc.sync.dma_start(wt[:], w[ts(kc, P), ts(u, n_tile)])
                        for ko in range(min(5, k_tiles)):
                            load_xt(ko)
                    else:
                        if u == 0 and kc + 4 < k_tiles:
                            load_xt(kc + 4)
                        nc.sync.dma_start(wt[:], w[ts(kc, P), ts(u, n_tile)])
                    if kc == max(k_tiles - 4, 0):
                        # this u's bias slice, shortly before its drain
                        nc.sync.dma_start(
                            bias_bc[:, ts(u, n_tile)],
                            bias[None, ts(u, n_tile)].to_broadcast([P, n_tile]),
                        )
                    st = sp.tile([P, n_tile], F16, tag="st")
                    nc.scalar.activation(
                        st[:], wt[:], mybir.ActivationFunctionType.Sign
                    )
                    for bt in range(b_tiles):
                        nc.tensor.matmul(
                            psums[bt][:],
                            xt[:, kc, ts(bt, P)],
                            st[:],
                            start=(kc == 0),
                            stop=(kc == k_tiles - 1),
                        )
                for bt in range(b_tiles):
                    ot = op.tile([P, n_tile], F32, tag="ot")
                    nc.vector.tensor_add(
                        ot[:], psums[bt][:], bias_bc[:, ts(u, n_tile)]
                    )
                    nc.scalar.dma_start(out[ts(bt, P), ts(u, n_tile)], ot[:])


def build_nc(rows=ROWS, d_in=D_IN, units=UNITS, n_tile=N_TILE):
    nc = bacc.Bacc(
        "TRN2", target_bir_lowering=False, debug=False, num_devices=N_CORES
    )
    xt = nc.dram_tensor("xt", [d_in, rows], F32, kind="ExternalInput").ap()
    w = nc.dram_tensor("w", [d_in, units], F32, kind="ExternalInput").ap()
    bias = nc.dram_tensor("bias", [units], F32, kind="ExternalInput").ap()
    out = nc.dram_tensor("out", [rows, units], F32, kind="ExternalOutput").ap()
    with tile.TileContext(nc) as tc:
        build_body(tc, xt, w, bias, out, rows, d_in, units, n_tile)
    nc.compile()
    return nc


_NC = None


def _get_nc():
    global _NC
    if _NC is None:
        _NC = build_nc()
    return _NC


def run_spmd(x, w, b, trace=False):
    nc = _get_nc()
    in_maps = [
        {
            "xt": np.ascontiguousarray(x[c * ROWS : (c + 1) * ROWS].T),
            "w": w,
            "bias": b,
        }
        for c in range(N_CORES)
    ]
    res = run_bass_kernel_spmd(
        nc, in_maps, core_ids=list(range(N_CORES)), trace=trace
    )
    out = np.concatenate([res.results[c]["out"] for c in range(N_CORES)], axis=0)
    return out, res


def kernel(x, kernel, bias):
    x = np.ascontiguousarray(x, dtype=np.float32)
    w = np.ascontiguousarray(kernel, dtype=np.float32)
    b = np.ascontiguousarray(bias, dtype=np.float32)
    out, _ = run_spmd(x, w, b)
    return out
